# revision 42
# baseline (speedup 1.0000x reference)
"""DGCNN encoder (4x GraphConv + SortPooling) as a dense-adjacency Bass kernel.

Sharding: graph-level data parallelism. 8 cores x 4 graphs each.
Host prep: edge lists -> per-graph dense normalized adjacency (bf16),
features transposed. All feature math runs on device; per-core [4,1024]
outputs are concatenated on host (outputs are disjoint, no collective).

v3 layout/schedule (on top of v2's dst-outer agg + activation chase):
- adjacency DMA split into 16 tiles per graph ([128, 4, 512] chunk
  groups) so layer-1 agg consumes it at 1MB granularity as it arrives.
- layer-1 W matmuls for ALL graphs run up front (prefill) while the
  first graph's adjacency streams; fills the cold-start PE idle window
  and removes the per-graph-boundary L1-W bubble.
- sortpooling runs per graph (top-16 select + gather + per-row sort +
  output DMA) overlapped under the next graph's compute; only the last
  graph's selection chain remains exposed at the tail.
"""

import os
import sys

import numpy as np

sys.path.insert(0, "/opt/trn_rl_repo")

import ml_dtypes  # noqa: E402

from concourse import bass, bacc, mybir, tile  # noqa: E402
from concourse import bass_utils  # noqa: E402

# Problem constants (hardcoded per spec; kernel.py must be self-contained).
B, N, DEG = 32, 2048, 16
IN_DIM = 128
HID = [128, 128, 128, 64]
K = 16
NCORES = 8
GPC = B // NCORES           # graphs per core = 4
NODES = GPC * N             # nodes per core = 8192
NCH = N // 128              # node chunks per graph = 16
NAT = 16                    # adjacency tiles per graph (4 chunks x 512 dst)

F32 = mybir.dt.float32
F16 = mybir.dt.bfloat16
I16 = mybir.dt.int16
I32 = mybir.dt.int32
U32 = mybir.dt.uint32

LAST = {"exec_time_ns": None}
_CACHE = {}


def _build_graph():
    """Build the per-core SPMD Bass graph (identical on all cores)."""
    nc = bacc.Bacc(
        "TRN2",
        target_bir_lowering=False,
        debug=False,
        enable_asserts=False,
        num_devices=NCORES,
    )

    featT = nc.dram_tensor("featT", [128, NODES], F16, kind="ExternalInput")
    # [graph, tile, 128 src-part, 4 chunks, 512 dst]
    at_in = nc.dram_tensor("at", [GPC, NAT, 128, 4, 512], F16, kind="ExternalInput")
    w_in = [
        nc.dram_tensor(f"w{i+1}", [128, HID[i]], F16, kind="ExternalInput")
        for i in range(4)
    ]
    b_in = [
        nc.dram_tensor(f"b{i+1}", [HID[i], 1], F32, kind="ExternalInput")
        for i in range(4)
    ]
    ident_in = nc.dram_tensor("ident", [128, 128], F32, kind="ExternalInput")
    rep_in = nc.dram_tensor("repmat", [16, 128], F32, kind="ExternalInput")
    out_dram = nc.dram_tensor("out", [GPC, K * 64], F32, kind="ExternalOutput")
    h4_hbm = nc.dram_tensor("h4hbm", [NODES, 64], F32)

    relu = mybir.ActivationFunctionType.Relu
    mxo = mybir.AluOpType.max

    with tile.TileContext(nc) as tc:
        with (
            tc.tile_pool(name="const", bufs=1) as constp,
            tc.tile_pool(name="atp", bufs=2 * NAT) as atp,
            tc.tile_pool(name="htp", bufs=8) as htp,
            tc.tile_pool(name="hn1p", bufs=GPC * 4) as hn1p,
            tc.tile_pool(name="hnp", bufs=8) as hnp,
            tc.tile_pool(name="h4np", bufs=2) as h4np,
            tc.tile_pool(name="sortp", bufs=1) as sortp,
            tc.tile_pool(name="h4tp", bufs=4) as h4tp,
            tc.tile_pool(name="aggps", bufs=3, space="PSUM") as aggps,
            tc.tile_pool(name="wps", bufs=3, space="PSUM") as wps,
            tc.tile_pool(name="tps", bufs=2, space="PSUM") as tps,
        ):
            # ---- loads ordered so graph-0 compute starts ASAP ----
            # sync (HWDGE) queue: W1 + featT graph-0 slice first (they gate
            # the first W matmuls), then graph-0 adjacency in 16 x 1MB tiles
            # so dst-outer agg starts after ~1MB.
            wt = []
            w0 = constp.tile([128, HID[0]], F16, tag="w0")
            nc.sync.dma_start(out=w0[:, :], in_=w_in[0][:, :])
            wt.append(w0)
            # featT as separate 512-col tiles per graph: tile-granular DMA
            # dependencies let the first W matmuls start on the first chunk.
            ftg = []
            f0 = [constp.tile([128, 512], F16, tag=f"ft0_{q}", name=f"ft0_{q}")
                  for q in range(4)]
            for q in range(4):
                nc.sync.dma_start(
                    out=f0[q][:, :],
                    in_=featT[:, q * 512 : (q + 1) * 512],
                )
            ftg.append(f0)

            at_tiles = [None] * GPC
            at0 = [atp.tile([128, 4, 512], F16, tag="atq", name=f"at0_{t}")
                   for t in range(NAT)]
            for t in range(NAT):
                nc.sync.dma_start(out=at0[t][:, :, :], in_=at_in[0, t])
            at_tiles[0] = at0

            # gpsimd queue: remaining featT slices first (they gate the W
            # prefill), then the small constants.
            for g in range(1, GPC):
                fg = [constp.tile([128, 512], F16, tag=f"ft{g}_{q}",
                                  name=f"ft{g}_{q}") for q in range(4)]
                for q in range(4):
                    nc.gpsimd.dma_start(
                        out=fg[q][:, :],
                        in_=featT[:, g * N + q * 512 : g * N + (q + 1) * 512],
                    )
                ftg.append(fg)
            for i in range(1, 4):
                w = constp.tile([128, HID[i]], F16, tag=f"w{i}")
                nc.gpsimd.dma_start(out=w[:, :], in_=w_in[i][:, :])
                wt.append(w)
            bt = []
            for i in range(4):
                bb = constp.tile([HID[i], 1], F32, tag=f"b{i}")
                nc.gpsimd.dma_start(out=bb[:, :], in_=b_in[i][:, :])
                bt.append(bb)
            ident = constp.tile([128, 128], F32, tag="ident")
            nc.gpsimd.dma_start(out=ident[:, :], in_=ident_in[:, :])
            repm = constp.tile([16, 128], F32, tag="repm")
            nc.gpsimd.dma_start(out=repm[:, :], in_=rep_in[:, :])

            # ---- sortpool state ----
            # node-id iota for mantissa-LSB key embedding: id[p,c] = p + 128c
            iotai = constp.tile([128, NCH], I32, tag="iotai")
            nc.gpsimd.iota(iotai[:, :], pattern=[[128, NCH]], base=0,
                           channel_multiplier=1)
            # per-graph row offsets g*N for the batched index transform
            offs4 = sortp.tile([4, 1], I32, tag="offs4")
            nc.gpsimd.iota(offs4[:, :], pattern=[[0, 1]], base=0,
                           channel_multiplier=N)
            offs4f = sortp.tile([4, 1], F32, tag="offs4f")
            nc.vector.tensor_copy(offs4f[:, :], offs4[:, :])
            idx16 = sortp.tile([128, 8], I16, tag="idx16")
            nc.vector.memset(idx16[:, :], -1)
            gath = sortp.tile([128, 1, 64], F32, tag="gath")
            # level-2 candidate rows, one partition per graph
            crow4 = constp.tile([GPC, 256], F32, tag="crow4")

            # Dummy gather at program start: forces the GPSIMD DGE microcode
            # library swap (UNLOAD/LOAD_LIB + ~8us drain) to happen during
            # the DMA-bound head instead of ahead of the real gather in the
            # tail. Reads h4hbm before any write (garbage, unused).
            dumi = sortp.tile([128, 1], I16, tag="dumi")
            nc.vector.memset(dumi[:, :], 0)
            dumg = sortp.tile([128, 1, 64], F32, tag="dumg")
            nc.gpsimd.dma_gather(
                dumg[:, :, :], h4_hbm[:, :], dumi[:, :],
                num_idxs=16, num_idxs_reg=16, elem_size=64,
            )

            # ---- layer-1 W prefill: hn1[g][cq] = (featT chunk).T @ W1 ----
            # Runs while graph-0 adjacency streams; PE is otherwise idle.
            hn1 = [[None] * 4 for _ in range(GPC)]
            for g in range(GPC):
                for cq in range(4):
                    wp = wps.tile([128, 4, HID[0]], F32, tag="wp")
                    for i in range(4):
                        nc.tensor.matmul(
                            wp[:, i, :],
                            ftg[g][cq][:, i * 128 : (i + 1) * 128],
                            wt[0][:, :],
                            start=True, stop=True,
                        )
                    hn = hn1p.tile([128, 4, HID[0]], F16, tag="hn1",
                                   name=f"hn1_{g}_{cq}")
                    nc.vector.tensor_copy(hn[:, :, :], wp[:, :, :])
                    hn1[g][cq] = hn

            for g in range(GPC):
                at = at_tiles[g]
                # prefetch next graph's adjacency behind this graph's tiles
                if g + 1 < GPC:
                    nxt = [atp.tile([128, 4, 512], F16, tag="atq",
                                    name=f"at{g+1}_{t}") for t in range(NAT)]
                    for t in range(NAT):
                        nc.sync.dma_start(out=nxt[t][:, :, :], in_=at_in[g + 1, t])
                    at_tiles[g + 1] = nxt

                hT_prev = None  # layer input, transposed [Din<=128, N] bf16
                for li in range(4):
                    dout = HID[li]
                    # ---- W matmul: h'n[c] = (hT chunk).T @ W (node-major) ----
                    if li == 0:
                        hn_tiles = hn1[g]
                    else:
                        hn_tiles = []
                        for cq in range(4):
                            wp = wps.tile([128, 4, dout], F32, tag="wp")
                            for i in range(4):
                                c = cq * 4 + i
                                lhsT = hT_prev[c // 4][
                                    :, (c % 4) * 128 : (c % 4 + 1) * 128
                                ]
                                nc.tensor.matmul(
                                    wp[:, i, :], lhsT, wt[li][:, :dout],
                                    start=True, stop=True,
                                )
                            hn = hnp.tile([128, 4, dout], F16, tag="hn")
                            nc.vector.tensor_copy(hn[:, :, :], wp[:, :, :])
                            hn_tiles.append(hn)

                    def hnc(c):
                        return hn_tiles[c // 4][:, c % 4, :]

                    # ---- aggregation: aggT[d, dst] += h'n[src] @ AT ----
                    # dst-slice outer so scalar activations chase each
                    # finished PSUM slice at tile granularity.
                    aggs = [aggps.tile([128, 512], F32, tag="aggs", name=f"aggs{d}")
                            for d in range(4)]
                    if li < 3:
                        hTs = [htp.tile([128, 512], F16, tag="hts", name=f"hts{d}")
                               for d in range(4)]
                    else:
                        hTs = [h4tp.tile([64, 512], F32, tag="h4ts", name=f"h4ts{d}")
                               for d in range(4)]
                    for dsp in range(4):
                        for c in range(NCH):
                            nc.tensor.matmul(
                                aggs[dsp][:dout, :],
                                hnc(c),
                                at[dsp * 4 + c // 4][:, c % 4, :],
                                start=(c == 0),
                                stop=(c == NCH - 1),
                            )
                        nc.scalar.activation(
                            hTs[dsp][:dout, :], aggs[dsp][:dout, :], relu,
                            bias=bt[li][:, :],
                        )
                    if li < 3:
                        hT_prev = hTs
                    else:
                        h4T = hTs

                # ---- layer-4 post: transpose to node-major, rowmax, HBM ----
                mcg = sortp.tile([128, NCH], F32, tag="mcg")
                for cq in range(4):
                    h4n = h4np.tile([128, 4, 64], F32, tag="h4n")
                    for i in range(4):
                        c = cq * 4 + i
                        tp = tps.tile([128, 128], F32, tag="tp")
                        nc.tensor.transpose(
                            tp[:, :64],
                            h4T[c // 4][:, (c % 4) * 128 : (c % 4 + 1) * 128],
                            ident[:64, :64],
                        )
                        nc.vector.tensor_copy(h4n[:, i, :], tp[:, :64])
                    csl = slice(cq * 4, cq * 4 + 4)
                    nc.vector.tensor_reduce(
                        mcg[:, csl], h4n[:, :, :], axis=mybir.AxisListType.X,
                        op=mxo,
                    )
                    nc.sync.dma_start(
                        out=h4_hbm[
                            g * N + cq * 512 : g * N + (cq + 1) * 512, :
                        ].rearrange("(c p) f -> p c f", p=128),
                        in_=h4n[:, :, :],
                    )

                # ---- per-graph: embed ids in mantissa LSBs, level-1 top-16
                # per 128-node block (small parallel-lane DVE ops only) ----
                emb = sortp.tile([128, NCH], F32, tag="emb")
                nc.vector.tensor_scalar(
                    emb[:, :].bitcast(I32), mcg[:, :].bitcast(I32), -2048,
                    None, op0=mybir.AluOpType.bitwise_and)
                nc.vector.tensor_tensor(
                    emb[:, :].bitcast(I32), emb[:, :].bitcast(I32),
                    iotai[:, :], op=mybir.AluOpType.bitwise_or)
                tpg = tps.tile([128, 128], F32, tag="tp")
                nc.tensor.transpose(tpg[:NCH, :], emb[:, :], ident[:, :])
                mtg = sortp.tile([NCH, 128], F32, tag="mtg")
                nc.vector.tensor_copy(mtg[:, :], tpg[:NCH, :])
                cand = sortp.tile([16, 16], F32, tag="cand")
                nc.vector.max(cand[:, 0:8], mtg[:, :])
                mt2 = sortp.tile([16, 128], F32, tag="mt2")
                nc.vector.match_replace(mt2[:, :], cand[:, 0:8], mtg[:, :],
                                        -1e30)
                nc.vector.max(cand[:, 8:16], mt2[:, :])
                nc.sync.dma_start(
                    out=crow4[g : g + 1, :].rearrange("o (c j) -> o c j", j=16),
                    in_=cand[:, :],
                )

            # ---- batched level-2: global top-16 per graph, id extraction ----
            top16 = sortp.tile([GPC, 16], F32, tag="top16")
            nc.vector.max(top16[:, 0:8], crow4[:, :])
            cr2 = sortp.tile([GPC, 256], F32, tag="cr2")
            nc.vector.match_replace(cr2[:, :], top16[:, 0:8], crow4[:, :], -1e30)
            nc.vector.max(top16[:, 8:16], cr2[:, :])

            ki = sortp.tile([GPC, 16], I32, tag="ki")
            nc.vector.tensor_scalar(
                ki[:, :], top16[:, :].bitcast(I32), 2047, None,
                op0=mybir.AluOpType.bitwise_and)
            idxf32 = sortp.tile([GPC, 16], F32, tag="idxf32")
            nc.vector.tensor_copy(idxf32[:, :], ki[:, :])
            idxo = sortp.tile([GPC, 16], F32, tag="idxo")
            nc.vector.tensor_scalar(
                idxo[:, :], idxf32[:, :], offs4f[:, :], None,
                op0=mybir.AluOpType.add
            )
            tpi = tps.tile([128, 128], F32, tag="tp")
            nc.tensor.transpose(tpi[:16, :GPC], idxo[:, :], ident[:GPC, :GPC])
            t1s = sortp.tile([16, GPC], F32, tag="t1s")
            nc.vector.tensor_copy(t1s[:, :], tpi[:16, :GPC])
            tpr = tps.tile([128, 128], F32, tag="tp")
            nc.tensor.matmul(tpr[:, :GPC], repm[:, :], t1s[:, :], start=True,
                             stop=True)
            nc.vector.tensor_copy(idx16[:, 0:GPC], tpr[:, :GPC])

            # ---- gather the 64 selected node rows from HBM ----
            nc.gpsimd.dma_gather(
                gath[:, :, :],
                h4_hbm[:, :],
                idx16[:, :],
                num_idxs=128,
                num_idxs_reg=64,
                elem_size=64,
            )

            # ---- ascending sort of 64 values per row via max8 rounds on -x ----
            neg = sortp.tile([64, 64], F32, tag="neg")
            nc.vector.tensor_scalar(
                neg[:, :], gath[:64, 0, :], -1.0, None, op0=mybir.AluOpType.mult
            )
            desc = sortp.tile([64, 64], F32, tag="desc")
            pp0 = sortp.tile([64, 64], F32, tag="pp0")
            pp1 = sortp.tile([64, 64], F32, tag="pp1")
            pp = [pp0, pp1]
            cur = neg
            for r in range(8):
                nc.vector.max(desc[:, r * 8 : (r + 1) * 8], cur[:, :])
                if r < 7:
                    nxt2 = pp[r % 2]
                    nc.vector.match_replace(
                        nxt2[:, :], desc[:, r * 8 : (r + 1) * 8], cur[:, :], -1e30
                    )
                    cur = nxt2
            asc = sortp.tile([64, 64], F32, tag="asc")
            nc.vector.tensor_scalar(
                asc[:, :], desc[:, :], -1.0, None, op0=mybir.AluOpType.mult
            )

            # ---- write output [4, 1024] ----
            nc.sync.dma_start(
                out=out_dram[:, :].rearrange("g (r f) -> (g r) f", f=64),
                in_=asc[:, :],
            )

    nc.compile()
    return nc


def _host_prep(inputs):
    """Shard + structural preprocessing: per-graph normalized dense adjacency."""
    feats = np.asarray(inputs["features"], np.float32)
    src = np.asarray(inputs["src"], np.int64)
    dst = np.asarray(inputs["dst"], np.int64)
    n_rand = B * N * DEG
    rs, rd = src[:n_rand], dst[:n_rand]

    ident = np.eye(128, dtype=np.float32)
    repmat = np.tile(np.eye(16, dtype=np.float32), (1, 8))  # [16, 128]
    in_maps = []
    for core in range(NCORES):
        at_core = np.empty((GPC, NAT, 128, 4, 512), dtype=ml_dtypes.bfloat16)
        for g in range(GPC):
            gb = core * GPC + g
            s = rs[gb * N * DEG : (gb + 1) * N * DEG] - gb * N
            d = rd[gb * N * DEG : (gb + 1) * N * DEG] - gb * N
            cnt = np.bincount(s * N + d, minlength=N * N).astype(np.float32)
            cnt = cnt.reshape(N, N)
            np.fill_diagonal(cnt, np.diagonal(cnt) + 1.0)  # self loops
            odeg = cnt.sum(axis=1)
            ideg = cnt.sum(axis=0)
            od = (1.0 / np.sqrt(np.maximum(odeg, 1.0))).astype(np.float32)
            idg = (1.0 / np.sqrt(np.maximum(ideg, 1.0))).astype(np.float32)
            a = (od[:, None] * cnt) * idg[None, :]
            # [src, dst] -> [NAT, 128, 4, 512]:
            # tile t = (dst-quarter q = t//4, chunk-group cg = t%4);
            # at[t, p, i, :] = a[(cg*4+i)*128 + p, q*512:(q+1)*512]
            at_core[g] = (
                a.reshape(4, 4, 128, 4, 512)   # [cg, ch, p, q, 512]
                .transpose(3, 0, 2, 1, 4)       # [q, cg, p, ch, 512]
                .reshape(NAT, 128, 4, 512)
                .astype(ml_dtypes.bfloat16)
            )
        fshard = np.ascontiguousarray(
            feats[core * NODES : (core + 1) * NODES].T
        ).astype(ml_dtypes.bfloat16)
        m = {"featT": fshard, "at": at_core, "ident": ident, "repmat": repmat}
        for i in range(4):
            m[f"w{i+1}"] = np.asarray(inputs[f"W{i+1}"], np.float32).astype(
                ml_dtypes.bfloat16
            )
            m[f"b{i+1}"] = np.asarray(inputs[f"b{i+1}"], np.float32).reshape(-1, 1)
        in_maps.append(m)
    return in_maps


def kernel(**inputs):
    if "nc" not in _CACHE:
        _CACHE["nc"] = _build_graph()
    nc = _CACHE["nc"]
    in_maps = _host_prep(inputs)
    trace = bool(int(os.environ.get("KERNEL_TRACE", "0")))
    res = bass_utils.run_bass_kernel_spmd(
        nc, in_maps, core_ids=list(range(NCORES)), trace=trace
    )
    LAST["exec_time_ns"] = res.exec_time_ns
    out = np.concatenate([res.results[i]["out"] for i in range(NCORES)], axis=0)
    return out.astype(np.float32)
